# revision 66
# baseline (speedup 1.0000x reference)
"""Trainium2 Bass kernel for nn_KATLayer (KAT basis-function layer).

out[b,o] = sum_{i,n} exp(-z^2) * (1 + erf(alpha*z/sqrt(2))) * w[i,o,n]
  z = (x[b,i] - c[i,o,n]) / (|sigma|+1e-8),  c = |scale|*mx_start + mx_train

Sharding: output dim O split across 8 cores (O_shard=64). Per core:
  partitions = i (4 chunks of 128), free = (o_local, n) = 1024 per tile,
  tiles processed in QUADS (4 consecutive b, same i-chunk) so elementwise
  and activation ops run at free=4096, amortizing fixed overheads.

Math (all intermediates fp16; validated ~5e-4 rel err vs the 2e-2 gate):
  zm = (c - x)*rinv     [= -z; the c - x subtraction is the only
                         cancellation-sensitive step (fp32 internal).
                         Two alternatives, mixed ~half/half to balance
                         engines: (b)-quads let ACT compute d = c - x via
                         Identity with per-partition bias -x (Identity is
                         in EVERY act table set -> no table switch), then
                         DVE zm = d*rinv16 (fp16 TT 2x). (c)-quads use a
                         DVE scalar_tensor_tensor (1x) directly.]
  DVE:  um = zm*A3      [A3 = alpha/sqrt(2), fp16 TT 2x; = -alpha*z/sqrt2]
  ACT:  e  = Derivative_Erf(zm)    [= 2/sqrt(pi)*exp(-z^2), even in z]
  DVE:  q  = e*wt       [in place on e; wt = w*sqrt(pi)/2]
  ACT:  t  = Erf(-um)   [in place on um; = erf(alpha*z/sqrt(2))]
  DVE:  r  = q*t        [in place on q]
  PE :  psum += onehot_b.T @ q (right after q) ; psum += onehot_b.T @ r
        (the "+1" of (1+erf) is absorbed by accumulating BOTH q and r
        streams in PSUM — no fp16 STT, which only has a 1x uop)
Final: DVE reduce over n: psum(32,64,16) -> (32,64); DMA out.

Per-k consts (rinv16, A3, wt) are read through stride-0 broadcast APs
across the quad dim. Derivative_Erf and Erf live in different ACT table
sets (~2.7us/switch), so quads are processed in groups with phase-batched
activations (2 switches per group), enforced with no-sync scheduler edges.
"""
import sys

sys.path.insert(0, "/opt/trn_rl_repo")
import math

import numpy as np

B, I, O, N = 32, 512, 512, 16
NCORES = 8
OS = O // NCORES          # 64 output dims per core
KC = I // 128             # 4 i-chunks
P = 128
Q = 4                     # b's per quad
GQ = 5                    # quads per activation-phase group (20 tiles)
INV_SQRT2 = 0.7071067811865476
SQRT_PI_2 = math.sqrt(math.pi) / 2.0

_CACHE = {}
LAST_RESULTS = None


def _build_nc(reps=1, GQ=GQ):
    import concourse.bacc as bacc
    import concourse.mybir as mybir
    from concourse import tile
    from concourse.tile_rust import add_dep_helper

    fp32 = mybir.dt.float32
    fp16 = mybir.dt.float16
    AF = mybir.ActivationFunctionType
    ALU = mybir.AluOpType

    nc = bacc.Bacc(
        "TRN2", target_bir_lowering=False, debug=False, num_devices=NCORES
    )
    c_d = nc.dram_tensor("c", [KC, P, OS, N], fp32, kind="ExternalInput")
    r_d = nc.dram_tensor("r", [KC, P, OS, N], fp16, kind="ExternalInput")
    a_d = nc.dram_tensor("a", [KC, P, OS, N], fp16, kind="ExternalInput")
    w_d = nc.dram_tensor("w", [KC, P, OS, N], fp16, kind="ExternalInput")
    x_d = nc.dram_tensor("x", [P, KC * B], fp32, kind="ExternalInput")
    oh_d = nc.dram_tensor("oh", [P, B, B], fp16, kind="ExternalInput")
    out_d = nc.dram_tensor("out", [B, OS], fp32, kind="ExternalOutput")

    with tile.TileContext(nc) as tc:
        with (
            tc.tile_pool(name="const", bufs=1) as cpool,
            tc.tile_pool(name="dp", bufs=3) as dpool,
            tc.tile_pool(name="zp", bufs=3) as zpool,
            tc.tile_pool(name="eq", bufs=GQ + 2) as eqpool,
            tc.tile_pool(name="tp", bufs=GQ + 2) as tpool,
            tc.tile_pool(name="psum", bufs=1, space="PSUM") as psp,
            tc.tile_pool(name="outp", bufs=1) as opool,
        ):
            # small tensors first so compute can start early; then const
            # chunks ordered by first use (k=0 before k=1, ...). xn (= -x,
            # the ACT Identity bias) is derived on-chip to avoid more DMAs.
            x_sb = cpool.tile([P, KC * B], fp32, tag="x")
            oh_sb = cpool.tile([P, B, B], fp16, tag="oh")
            nc.sync.dma_start(x_sb[:], x_d[:])
            c_sb, r_sb, a_sb, w_sb = [], [], [], []
            for k in range(KC):
                for lst, dram, nm, dt_ in (
                    (c_sb, c_d, "c", fp32),
                    (r_sb, r_d, "r", fp16),
                    (a_sb, a_d, "a", fp16),
                    (w_sb, w_d, "w", fp16),
                ):
                    t = cpool.tile([P, OS, N], dt_, tag=f"{nm}{k}")
                    lst.append(t)
                nc.sync.dma_start(c_sb[k][:], c_d[k])
                nc.sync.dma_start(r_sb[k][:], r_d[k])
                if k == 0:
                    nc.sync.dma_start(oh_sb[:], oh_d[:])
                nc.sync.dma_start(a_sb[k][:], a_d[k])
                nc.sync.dma_start(w_sb[k][:], w_d[k])

            psum_t = psp.tile([B, OS, N], fp32)
            # quads: 4 consecutive b, same k
            quads = [(k, b) for k in range(KC) for b in range(0, B, Q)]
            n_quads = len(quads)
            # ~15/32 of quads compute d = c - x on ACT (engine balance);
            # first and last quads stay on DVE so the startup ramp and the
            # tail drain don't wait on the ACT Identity chain
            act_d = [(qi % 2 == 1 and qi < 28) or qi in (28, 30)
                     for qi in range(n_quads)]

            out_sb = opool.tile([B, OS], fp32)

            def bcastq(t):
                return t[:, None].broadcast_to((P, Q, OS, N))

            for rep in range(reps):
                n_mm = 0
                total_mm = 2 * Q * 2 * n_quads
                prev_erf = None
                sizes = [3, GQ, GQ, GQ, GQ, GQ, 4]
                assert sum(sizes) == n_quads
                bounds = []
                g0 = 0
                for s in sizes:
                    bounds.append((g0, g0 + s))
                    g0 += s
                for g0, g1 in bounds:
                    grp = list(range(g0, g1))
                    # phase 1: (b)-quads: d = x - c via ACT Identity
                    # (scale=-1, bias=+x). Sign flips vs the (c)-quads'
                    # zm = (c-x)*rinv; compensated by the Erf input scale.
                    ds = {}
                    for qi in grp:
                        k, b = quads[qi]
                        if not act_d[qi]:
                            continue
                        d = dpool.tile([P, Q, OS, N], fp16, tag="dp")
                        for j in range(Q):
                            col = slice(k * B + b + j, k * B + b + j + 1)
                            nc.scalar.activation(
                                d[:, j], c_sb[k][:], AF.Identity,
                                bias=x_sb[:, col], scale=-1.0,
                            )
                        ds[qi] = d
                    # phase 2: zm quads; um = zm*A3 (DVE)
                    ums, zms = {}, {}
                    for qi in grp:
                        k, b = quads[qi]
                        zq = zpool.tile([P, Q, OS, N], fp16, tag="zp")
                        if act_d[qi]:
                            nc.vector.tensor_mul(
                                zq[:], ds[qi][:], bcastq(r_sb[k])
                            )
                        else:
                            for j in range(Q):
                                col = slice(k * B + b + j, k * B + b + j + 1)
                                nc.vector.scalar_tensor_tensor(
                                    zq[:, j], c_sb[k][:], x_sb[:, col],
                                    r_sb[k][:],
                                    op0=ALU.subtract, op1=ALU.mult,
                                )
                        um = tpool.tile([P, Q, OS, N], fp16, tag="tp")
                        nc.vector.tensor_mul(um[:], zq[:], bcastq(a_sb[k]))
                        zms[qi], ums[qi] = zq, um
                    # phase 3: e = D_ERF(zm) (ACT table A); q = e*wt in
                    # place (DVE); q-stream matmuls (PE)
                    qs = {}
                    derfs = []
                    for qi in grp:
                        k, b = quads[qi]
                        e = eqpool.tile([P, Q, OS, N], fp16, tag="eq")
                        ei = nc.scalar.activation(
                            e[:], zms[qi][:], AF.Derivative_Erf
                        )
                        if prev_erf is not None:
                            add_dep_helper(ei.ins, prev_erf.ins, sync=False,
                                           reason="act table phase order")
                        derfs.append(ei)
                        nc.vector.tensor_mul(e[:], e[:], bcastq(w_sb[k]))
                        qs[qi] = e
                        for j in range(Q):
                            for h in range(2):
                                nc.tensor.matmul(
                                    psum_t[:, 32 * h : 32 * (h + 1), :],
                                    oh_sb[:, b + j, :],
                                    e[:, j, 32 * h : 32 * (h + 1), :],
                                    start=(n_mm < 2),
                                    stop=(n_mm >= total_mm - 2),
                                )
                                n_mm += 1
                    # phase 4: t = erf(alpha*z/sqrt2) in place (ACT table B):
                    # um = -alpha*z/sqrt2 for (c)-quads (scale=-1) but
                    # +alpha*z/sqrt2 for (b)-quads (scale=+1)
                    for qi in grp:
                        um = ums[qi]
                        ti = nc.scalar.activation(
                            um[:], um[:], AF.Erf,
                            scale=(1.0 if act_d[qi] else -1.0),
                        )
                        add_dep_helper(ti.ins, derfs[-1].ins, sync=False,
                                       reason="act table phase order")
                        prev_erf = ti
                    # phase 5: r = q*t in place on q (DVE, after q's
                    # matmuls); r-stream matmuls (PE)
                    for qi in grp:
                        k, b = quads[qi]
                        q_, t_ = qs[qi], ums[qi]
                        nc.vector.tensor_mul(q_[:], q_[:], t_[:])
                        for j in range(Q):
                            for h in range(2):
                                nc.tensor.matmul(
                                    psum_t[:, 32 * h : 32 * (h + 1), :],
                                    oh_sb[:, b + j, :],
                                    q_[:, j, 32 * h : 32 * (h + 1), :],
                                    start=(n_mm < 2),
                                    stop=(n_mm >= total_mm - 2),
                                )
                                n_mm += 1

            nc.vector.tensor_reduce(
                out_sb[:], psum_t[:], axis=mybir.AxisListType.X, op=ALU.add
            )
            nc.scalar.dma_start(out_d[:], out_sb[:])

    nc.compile()
    return nc


def _prep_inputs(x, mx_train, scale, sigma, alpha, w, mx_start):
    c = (np.abs(scale)[:, :, None] * mx_start[None, None, :]
         + mx_train[:, :, None]).astype(np.float32)
    rinv = (1.0 / (np.abs(sigma) + 1e-8)).astype(np.float32)
    r16 = rinv.astype(np.float16)
    A3 = (alpha * INV_SQRT2).astype(np.float16)
    wt = (w * SQRT_PI_2).astype(np.float16)
    # x packed as [P, KC*B]: xp[p, k*B+b] = x[b, k*128+p]
    xp = np.ascontiguousarray(
        x.T.reshape(KC, P, B).transpose(1, 0, 2).reshape(P, KC * B)
    ).astype(np.float32)
    oh = np.broadcast_to(np.eye(B, dtype=np.float16), (P, B, B))
    oh = np.ascontiguousarray(oh)

    in_maps = []
    for d in range(NCORES):
        sl = slice(d * OS, (d + 1) * OS)
        in_maps.append({
            "c": np.ascontiguousarray(c[:, sl].reshape(KC, P, OS, N)),
            "r": np.ascontiguousarray(r16[:, sl].reshape(KC, P, OS, N)),
            "a": np.ascontiguousarray(A3[:, sl].reshape(KC, P, OS, N)),
            "w": np.ascontiguousarray(wt[:, sl].reshape(KC, P, OS, N)),
            "x": xp,
            "oh": oh,
        })
    return in_maps


def kernel(x, mx_train, scale, sigma, alpha, w, mx_start, _trace=False):
    global LAST_RESULTS
    from concourse.bass_utils import run_bass_kernel_spmd

    if "nc" not in _CACHE:
        _CACHE["nc"] = _build_nc()
    nc = _CACHE["nc"]
    in_maps = _prep_inputs(
        np.asarray(x, np.float32), np.asarray(mx_train, np.float32),
        np.asarray(scale, np.float32), np.asarray(sigma, np.float32),
        np.asarray(alpha, np.float32), np.asarray(w, np.float32),
        np.asarray(mx_start, np.float32),
    )
    res = run_bass_kernel_spmd(nc, in_maps, core_ids=list(range(NCORES)),
                               trace=_trace)
    LAST_RESULTS = res
    return np.concatenate([r["out"] for r in res.results], axis=1)


# revision 67
# speedup vs baseline: 1.0025x; 1.0025x over previous
"""Trainium2 Bass kernel for nn_KATLayer (KAT basis-function layer).

out[b,o] = sum_{i,n} exp(-z^2) * (1 + erf(alpha*z/sqrt(2))) * w[i,o,n]
  z = (x[b,i] - c[i,o,n]) / (|sigma|+1e-8),  c = |scale|*mx_start + mx_train

Sharding: output dim O split across 8 cores (O_shard=64). Per core:
  partitions = i (4 chunks of 128), free = (o_local, n) = 1024 per tile,
  tiles processed in QUADS (4 consecutive b, same i-chunk) so elementwise
  and activation ops run at free=4096, amortizing fixed overheads.

Math (all intermediates fp16; validated ~5e-4 rel err vs the 2e-2 gate):
  zm = (c - x)*rinv     [= -z; the c - x subtraction is the only
                         cancellation-sensitive step (fp32 internal).
                         Two alternatives, mixed ~half/half to balance
                         engines: (b)-quads let ACT compute d = c - x via
                         Identity with per-partition bias -x (Identity is
                         in EVERY act table set -> no table switch), then
                         DVE zm = d*rinv16 (fp16 TT 2x). (c)-quads use a
                         DVE scalar_tensor_tensor (1x) directly.]
  DVE:  um = zm*A3      [A3 = alpha/sqrt(2), fp16 TT 2x; = -alpha*z/sqrt2]
  ACT:  e  = Derivative_Erf(zm)    [= 2/sqrt(pi)*exp(-z^2), even in z]
  DVE:  q  = e*wt       [in place on e; wt = w*sqrt(pi)/2]
  ACT:  t  = Erf(-um)   [in place on um; = erf(alpha*z/sqrt(2))]
  DVE:  r  = q*t        [in place on q]
  PE :  psum += onehot_b.T @ q (right after q) ; psum += onehot_b.T @ r
        (the "+1" of (1+erf) is absorbed by accumulating BOTH q and r
        streams in PSUM — no fp16 STT, which only has a 1x uop)
Final: DVE reduce over n: psum(32,64,16) -> (32,64); DMA out.

Per-k consts (rinv16, A3, wt) are read through stride-0 broadcast APs
across the quad dim. Derivative_Erf and Erf live in different ACT table
sets (~2.7us/switch), so quads are processed in groups with phase-batched
activations (2 switches per group), enforced with no-sync scheduler edges.
"""
import sys

sys.path.insert(0, "/opt/trn_rl_repo")
import math

import numpy as np

B, I, O, N = 32, 512, 512, 16
NCORES = 8
OS = O // NCORES          # 64 output dims per core
KC = I // 128             # 4 i-chunks
P = 128
Q = 4                     # b's per quad
GQ = 5                    # quads per activation-phase group (20 tiles)
INV_SQRT2 = 0.7071067811865476
SQRT_PI_2 = math.sqrt(math.pi) / 2.0

_CACHE = {}
LAST_RESULTS = None


def _build_nc(reps=1, GQ=GQ):
    import concourse.bacc as bacc
    import concourse.mybir as mybir
    from concourse import tile
    from concourse.tile_rust import add_dep_helper

    fp32 = mybir.dt.float32
    fp16 = mybir.dt.float16
    AF = mybir.ActivationFunctionType
    ALU = mybir.AluOpType

    nc = bacc.Bacc(
        "TRN2", target_bir_lowering=False, debug=False, num_devices=NCORES
    )
    c_d = nc.dram_tensor("c", [KC, P, OS, N], fp32, kind="ExternalInput")
    r_d = nc.dram_tensor("r", [KC, P, OS, N], fp16, kind="ExternalInput")
    a_d = nc.dram_tensor("a", [KC, P, OS, N], fp16, kind="ExternalInput")
    w_d = nc.dram_tensor("w", [KC, P, OS, N], fp16, kind="ExternalInput")
    x_d = nc.dram_tensor("x", [P, KC * B], fp32, kind="ExternalInput")
    oh_d = nc.dram_tensor("oh", [P, B, B], fp16, kind="ExternalInput")
    out_d = nc.dram_tensor("out", [B, OS], fp32, kind="ExternalOutput")

    with tile.TileContext(nc) as tc:
        with (
            tc.tile_pool(name="const", bufs=1) as cpool,
            tc.tile_pool(name="dp", bufs=3) as dpool,
            tc.tile_pool(name="zp", bufs=3) as zpool,
            tc.tile_pool(name="eq", bufs=GQ + 2) as eqpool,
            tc.tile_pool(name="tp", bufs=GQ + 2) as tpool,
            tc.tile_pool(name="psum", bufs=1, space="PSUM") as psp,
            tc.tile_pool(name="outp", bufs=1) as opool,
        ):
            # small tensors first so compute can start early; then const
            # chunks ordered by first use (k=0 before k=1, ...). xn (= -x,
            # the ACT Identity bias) is derived on-chip to avoid more DMAs.
            x_sb = cpool.tile([P, KC * B], fp32, tag="x")
            oh_sb = cpool.tile([P, B, B], fp16, tag="oh")
            nc.sync.dma_start(x_sb[:], x_d[:])
            c_sb, r_sb, a_sb, w_sb = [], [], [], []
            for k in range(KC):
                for lst, dram, nm, dt_ in (
                    (c_sb, c_d, "c", fp32),
                    (r_sb, r_d, "r", fp16),
                    (a_sb, a_d, "a", fp16),
                    (w_sb, w_d, "w", fp16),
                ):
                    t = cpool.tile([P, OS, N], dt_, tag=f"{nm}{k}")
                    lst.append(t)
                nc.sync.dma_start(c_sb[k][:], c_d[k])
                nc.sync.dma_start(r_sb[k][:], r_d[k])
                if k == 0:
                    nc.sync.dma_start(oh_sb[:], oh_d[:])
                nc.sync.dma_start(a_sb[k][:], a_d[k])
                nc.sync.dma_start(w_sb[k][:], w_d[k])

            psum_t = psp.tile([B, OS, N], fp32)
            # quads: 4 consecutive b, same k
            quads = [(k, b) for k in range(KC) for b in range(0, B, Q)]
            n_quads = len(quads)
            # ~15/32 of quads compute d = c - x on ACT (engine balance);
            # first and last quads stay on DVE so the startup ramp and the
            # tail drain don't wait on the ACT Identity chain
            act_d = [(qi % 2 == 1 and qi < 28) or qi in (28, 30)
                     for qi in range(n_quads)]

            out_sb = opool.tile([B, OS], fp32)

            def bcastq(t):
                return t[:, None].broadcast_to((P, Q, OS, N))

            for rep in range(reps):
                n_mm = 0
                total_mm = 2 * Q * 2 * n_quads
                prev_erf = None
                sizes = [GQ] * 5 + [4, 3]
                assert sum(sizes) == n_quads
                bounds = []
                g0 = 0
                for s in sizes:
                    bounds.append((g0, g0 + s))
                    g0 += s
                for g0, g1 in bounds:
                    grp = list(range(g0, g1))
                    # phase 1: (b)-quads: d = x - c via ACT Identity
                    # (scale=-1, bias=+x). Sign flips vs the (c)-quads'
                    # zm = (c-x)*rinv; compensated by the Erf input scale.
                    ds = {}
                    for qi in grp:
                        k, b = quads[qi]
                        if not act_d[qi]:
                            continue
                        d = dpool.tile([P, Q, OS, N], fp16, tag="dp")
                        for j in range(Q):
                            col = slice(k * B + b + j, k * B + b + j + 1)
                            nc.scalar.activation(
                                d[:, j], c_sb[k][:], AF.Identity,
                                bias=x_sb[:, col], scale=-1.0,
                            )
                        ds[qi] = d
                    # phase 2: zm quads; um = zm*A3 (DVE)
                    ums, zms = {}, {}
                    for qi in grp:
                        k, b = quads[qi]
                        zq = zpool.tile([P, Q, OS, N], fp16, tag="zp")
                        if act_d[qi]:
                            nc.vector.tensor_mul(
                                zq[:], ds[qi][:], bcastq(r_sb[k])
                            )
                        else:
                            for j in range(Q):
                                col = slice(k * B + b + j, k * B + b + j + 1)
                                nc.vector.scalar_tensor_tensor(
                                    zq[:, j], c_sb[k][:], x_sb[:, col],
                                    r_sb[k][:],
                                    op0=ALU.subtract, op1=ALU.mult,
                                )
                        um = tpool.tile([P, Q, OS, N], fp16, tag="tp")
                        nc.vector.tensor_mul(um[:], zq[:], bcastq(a_sb[k]))
                        zms[qi], ums[qi] = zq, um
                    # phase 3: e = D_ERF(zm) (ACT table A); q = e*wt in
                    # place (DVE); q-stream matmuls (PE)
                    qs = {}
                    derfs = []
                    for qi in grp:
                        k, b = quads[qi]
                        e = eqpool.tile([P, Q, OS, N], fp16, tag="eq")
                        ei = nc.scalar.activation(
                            e[:], zms[qi][:], AF.Derivative_Erf
                        )
                        if prev_erf is not None:
                            add_dep_helper(ei.ins, prev_erf.ins, sync=False,
                                           reason="act table phase order")
                        derfs.append(ei)
                        nc.vector.tensor_mul(e[:], e[:], bcastq(w_sb[k]))
                        qs[qi] = e
                        for j in range(Q):
                            for h in range(2):
                                nc.tensor.matmul(
                                    psum_t[:, 32 * h : 32 * (h + 1), :],
                                    oh_sb[:, b + j, :],
                                    e[:, j, 32 * h : 32 * (h + 1), :],
                                    start=(n_mm < 2),
                                    stop=(n_mm >= total_mm - 2),
                                )
                                n_mm += 1
                    # phase 4: t = erf(alpha*z/sqrt2) in place (ACT table B):
                    # um = -alpha*z/sqrt2 for (c)-quads (scale=-1) but
                    # +alpha*z/sqrt2 for (b)-quads (scale=+1)
                    for qi in grp:
                        um = ums[qi]
                        ti = nc.scalar.activation(
                            um[:], um[:], AF.Erf,
                            scale=(1.0 if act_d[qi] else -1.0),
                        )
                        add_dep_helper(ti.ins, derfs[-1].ins, sync=False,
                                       reason="act table phase order")
                        prev_erf = ti
                    # phase 5: r = q*t in place on q (DVE, after q's
                    # matmuls); r-stream matmuls (PE)
                    for qi in grp:
                        k, b = quads[qi]
                        q_, t_ = qs[qi], ums[qi]
                        nc.vector.tensor_mul(q_[:], q_[:], t_[:])
                        for j in range(Q):
                            for h in range(2):
                                nc.tensor.matmul(
                                    psum_t[:, 32 * h : 32 * (h + 1), :],
                                    oh_sb[:, b + j, :],
                                    q_[:, j, 32 * h : 32 * (h + 1), :],
                                    start=(n_mm < 2),
                                    stop=(n_mm >= total_mm - 2),
                                )
                                n_mm += 1

            nc.vector.tensor_reduce(
                out_sb[:], psum_t[:], axis=mybir.AxisListType.X, op=ALU.add
            )
            nc.scalar.dma_start(out_d[:], out_sb[:])

    nc.compile()
    return nc


def _prep_inputs(x, mx_train, scale, sigma, alpha, w, mx_start):
    c = (np.abs(scale)[:, :, None] * mx_start[None, None, :]
         + mx_train[:, :, None]).astype(np.float32)
    rinv = (1.0 / (np.abs(sigma) + 1e-8)).astype(np.float32)
    r16 = rinv.astype(np.float16)
    A3 = (alpha * INV_SQRT2).astype(np.float16)
    wt = (w * SQRT_PI_2).astype(np.float16)
    # x packed as [P, KC*B]: xp[p, k*B+b] = x[b, k*128+p]
    xp = np.ascontiguousarray(
        x.T.reshape(KC, P, B).transpose(1, 0, 2).reshape(P, KC * B)
    ).astype(np.float32)
    oh = np.broadcast_to(np.eye(B, dtype=np.float16), (P, B, B))
    oh = np.ascontiguousarray(oh)

    in_maps = []
    for d in range(NCORES):
        sl = slice(d * OS, (d + 1) * OS)
        in_maps.append({
            "c": np.ascontiguousarray(c[:, sl].reshape(KC, P, OS, N)),
            "r": np.ascontiguousarray(r16[:, sl].reshape(KC, P, OS, N)),
            "a": np.ascontiguousarray(A3[:, sl].reshape(KC, P, OS, N)),
            "w": np.ascontiguousarray(wt[:, sl].reshape(KC, P, OS, N)),
            "x": xp,
            "oh": oh,
        })
    return in_maps


def kernel(x, mx_train, scale, sigma, alpha, w, mx_start, _trace=False):
    global LAST_RESULTS
    from concourse.bass_utils import run_bass_kernel_spmd

    if "nc" not in _CACHE:
        _CACHE["nc"] = _build_nc()
    nc = _CACHE["nc"]
    in_maps = _prep_inputs(
        np.asarray(x, np.float32), np.asarray(mx_train, np.float32),
        np.asarray(scale, np.float32), np.asarray(sigma, np.float32),
        np.asarray(alpha, np.float32), np.asarray(w, np.float32),
        np.asarray(mx_start, np.float32),
    )
    res = run_bass_kernel_spmd(nc, in_maps, core_ids=list(range(NCORES)),
                               trace=_trace)
    LAST_RESULTS = res
    return np.concatenate([r["out"] for r in res.results], axis=1)
